# revision 43
# baseline (speedup 1.0000x reference)
"""Differential attention (DiffAttn) Trainium2 kernel, 8-core tensor-parallel.

Reference computation (per batch b, head h):
    q1,k1,q2,k2,v = x @ W*.T          (x: [B,S,D], W: [D,D], 16 heads x 128)
    a1 = softmax(q1 k1^T / sqrt(dh)); a2 = softmax(q2 k2^T / sqrt(dh))
    out = ((a1 - lam_h * a2) @ v) @ o_w.T

Sharding: tensor-parallel over heads. Core c owns heads {2c, 2c+1} (d_model
slice 256c:256c+256 of the projection outputs).  Each core computes a partial
o-projection output over its 256 input dims; the host sums the 8 partials.

Device-side layout choices:
  - x is passed pre-transposed (xt = x.T, [D, B*S]) so projections can run
    as  out.T[m, tok] = W_shard @ x.T  with the weight shard (host
    pre-transposed) as the stationary operand -> q/k tiles land in
    [head_dim(part), token(free)] layout, which feeds QK^T directly.
  - v is produced in natural [token, dim] layout (lhsT = x.T chunks) so it can
    be the stationary operand of the PV matmul.
  - probabilities are transposed [q,k]->[k,q] on the PE in 128x128 blocks
    (batched 8 per PSUM bank); softmax row-sums ride the exp activation's
    accumulator; normalization is two DVE passes (tensor_scalar + fused STT).
  - all matmul inputs are bf16 (measured |rel err| ~5e-3 end to end vs the
    fp32 reference); PSUM accumulation is fp32; softmax stats are fp32.

Engine balance: attention is elementwise-bound (ACT exp, DVE normalize), the
projections are PE-bound.  The emission order software-pipelines them:

    proj(b0,h0)+v(b0) | attn(b0,h0) x proj(b0,h1) | attn(b0,h1) x proj(b1,h0)+v(b1)
    | attn(b1,h0) x proj(b1,h1) | attn(b1,h1) x oproj(b0) | oproj(b1)

Engines consume their instruction streams in order, so interleaved emission is
what lets the PE run projection matmuls while ACT/DVE chew on attention; the
SBUF slots are shared across batches (same tags), so Tile's WAR deps give the
handoff for free.
"""

import math

import numpy as np
import ml_dtypes

import concourse.bass as bass
import concourse.mybir as mybir
import concourse.tile as tile
from concourse import bass_utils
from concourse.masks import make_identity

BF16 = mybir.dt.bfloat16
F32 = mybir.dt.float32

P = 128           # partitions / head_dim / PE tile
D = 2048          # d_model
B = 2
S = 2048          # seq len
T = B * S         # 4096 tokens
NH = 16           # total heads
NHL = 2           # heads per core
MD = NHL * P      # per-core projection dim (256)
KT = D // P       # 16 contraction tiles over d_model
ST = S // P       # 16 token tiles per batch
N_CORES = 8
CHUNK = 512       # token chunk for projection x streaming
QC = 512          # q-chunk for the PV stage
SCALE = 1.0 / math.sqrt(P)
QKN = ["wq1", "wk1", "wq2", "wk2"]

_mult = mybir.AluOpType.mult
_add = mybir.AluOpType.add


def _split_multi_waits(nc):
    """This walrus build accepts at most ONE sync-wait per instruction
    (codegen: "Too many sync wait commands").  Tile attaches one wait per
    upstream proc, so split the extras onto same-engine NOP carriers placed
    immediately before the instruction — the engine stalls on each carrier in
    turn, which is sequentially equivalent."""
    n = 0
    for bb in nc.main_func.blocks:
        out = []
        for ins in bb.instructions:
            si = getattr(ins, "sync_info", None)
            waits = list(si.on_wait) if si is not None and si.on_wait else []
            if len(waits) > 1:
                for w in waits[:-1]:
                    n += 1
                    out.append(
                        mybir.InstNoOp(
                            name=f"{ins.name}-wsplit{n}",
                            engine=ins.engine,
                            sync_info=mybir.SyncInfo(on_wait=[w], on_update=[]),
                            bass_nofuse=True,
                        )
                    )
                si.on_wait = waits[-1:]
            out.append(ins)
        bb.instructions[:] = out


class Kern:
    """Holds pools/constants; methods emit one group of instructions each.
    The driver (build) calls them in a software-pipelined order."""

    def __init__(self, nc, tc, pools):
        self.nc = nc
        self.tc = tc
        (self.cpool, self.projpool, self.xpool, self.apool, self.ptpool,
         self.ps_score, self.ps_mm, self.ps_tp) = pools
        self.qk = {}      # (b, n, h) -> tile (slots shared across b via tags)
        self.vbuf = {}    # b -> tile
        self.aoT = {}     # (b, h) -> tile
        self.xc_cur = None
        self.w_sb = {}

    def load_w(self, w_d, n, split=False):
        t = self.cpool.tile([P, KT, MD], BF16, name=f"{n}_sb")
        src_ap = w_d[n].rearrange("p (kt m) -> p kt m", m=MD)
        if split:
            self.nc.sync.dma_start(t[:, : KT // 2], src_ap[:, : KT // 2])
            self.nc.sync.dma_start(t[:, KT // 2 :], src_ap[:, KT // 2 :])
        else:
            self.nc.sync.dma_start(t, src_ap)
        self.w_sb[n] = t

    def load_consts(self, neglam_d):
        nc = self.nc
        self.neglam_sb = self.cpool.tile([P, NHL], F32, name="neglam_sb")
        nc.sync.dma_start(self.neglam_sb, neglam_d.ap())
        self.ident = self.cpool.tile([P, P], BF16, name="ident")
        make_identity(nc, self.ident)

    def load_wo(self, wo_d):
        self.wo_sb = self.cpool.tile([P, NHL, D], BF16, name="wo_sb")
        self.nc.sync.dma_start(self.wo_sb, wo_d.rearrange("p (h n) -> p h n", n=D))

    # ---- projection pieces ----
    def load_xc(self, xt, b, ci, half=None):
        tok0 = b * S + ci * CHUNK
        if half is None:
            xc = self.xpool.tile([P, KT, CHUNK], BF16, name="xc", tag="xc")
            self.nc.sync.dma_start(
                xc, xt[:, tok0 : tok0 + CHUNK].rearrange("(kt p) n -> p kt n", p=P)
            )
            self.xc_cur = xc
        else:
            # first-chunk fast path: land each half as its own DMA so the
            # first matmuls only wait on 1MB of x
            if half == 0:
                self.xc_cur = self.xpool.tile([P, KT, CHUNK], BF16, name="xc", tag="xc")
            hw = CHUNK // 2
            self.nc.sync.dma_start(
                self.xc_cur[:, :, half * hw : (half + 1) * hw],
                xt[:, tok0 + half * hw : tok0 + (half + 1) * hw].rearrange(
                    "(kt p) n -> p kt n", p=P
                ),
            )

    def proj_qk_group(self, b, n, h, ci, half=None):
        """One (weight, head) projection for one 512-token chunk: 16 matmuls
        accumulated in PSUM, then a DVE cast into the persistent qk tile."""
        nc = self.nc
        key = (b, n, h)
        if key not in self.qk:
            self.qk[key] = self.projpool.tile(
                [P, S], BF16, name=f"{n}h{h}b{b}", tag=f"{n}h{h}"
            )
        ps = self.ps_mm.tile([P, 512], F32, name="psp", tag="mm")
        xc = self.xc_cur
        lo, sz = (0, CHUNK) if half is None else (half * (CHUNK // 2), CHUNK // 2)
        for kt in range(KT):
            nc.tensor.matmul(
                ps[:, :sz],
                lhsT=self.w_sb[n][:, kt, h * P : (h + 1) * P],
                rhs=xc[:, kt, lo : lo + sz],
                start=(kt == 0),
                stop=(kt == KT - 1),
            )
        nc.vector.tensor_copy(
            self.qk[key][:, ci * CHUNK + lo : ci * CHUNK + lo + sz], ps[:, :sz]
        )

    def proj_v_group(self, b, ci, s4):
        nc = self.nc
        if b not in self.vbuf:
            self.vbuf[b] = self.projpool.tile(
                [P, ST, MD], BF16, name=f"vbuf{b}", tag=f"vbuf{b}"
            )
        ps = self.ps_mm.tile([P, 512], F32, name="psv", tag="mm")
        xc = self.xc_cur
        for kt in range(KT):
            nc.tensor.matmul(
                ps[:, :MD],
                lhsT=xc[:, kt, s4 * P : (s4 + 1) * P],
                rhs=self.w_sb["wv"][:, kt, :],
                start=(kt == 0),
                stop=(kt == KT - 1),
            )
        nc.vector.tensor_copy(
            self.vbuf[b][:, ci * (CHUNK // P) + s4, :], ps[:, :MD]
        )

    # ---- attention pieces ----
    def attn_qtile(self, b, h, qc, qt4, ptbuf):
        nc = self.nc
        apool = self.apool
        qi = qc * QC + qt4 * P
        q1h, k1h = self.qk[(b, "wq1", h)], self.qk[(b, "wk1", h)]
        q2h, k2h = self.qk[(b, "wq2", h)], self.qk[(b, "wk2", h)]
        ebufs, rstats = [], []
        for si, (qh, kh) in enumerate([(q1h, k1h), (q2h, k2h)]):
            e = apool.tile([P, S], BF16, name=f"e{si}", tag=f"e{si}")
            ssc = apool.tile([P, 2], F32, name=f"ssc{si}", tag=f"ssc{si}")
            for hf in range(2):
                ps = self.ps_score.tile([P, 1024], F32, name="pss", tag="score")
                for j in range(2):
                    nc.tensor.matmul(
                        ps[:, j * 512 : (j + 1) * 512],
                        lhsT=qh[:, qi : qi + P],
                        rhs=kh[:, hf * 1024 + j * 512 : hf * 1024 + (j + 1) * 512],
                        start=True,
                        stop=True,
                    )
                nc.scalar.activation(
                    e[:, hf * 1024 : (hf + 1) * 1024],
                    ps,
                    mybir.ActivationFunctionType.Exp,
                    scale=SCALE,
                    accum_out=ssc[:, hf : hf + 1],
                )
            ebufs.append(e)
            rstats.append(ssc)
        s1 = apool.tile([P, 1], F32, name="s1", tag="s1")
        nc.vector.tensor_add(s1, rstats[0][:, 0:1], rstats[0][:, 1:2])
        r1 = apool.tile([P, 1], F32, name="r1", tag="r1")
        nc.vector.reciprocal(r1, s1)
        s2 = apool.tile([P, 1], F32, name="s2", tag="s2")
        nc.vector.tensor_add(s2, rstats[1][:, 0:1], rstats[1][:, 1:2])
        r2 = apool.tile([P, 1], F32, name="r2", tag="r2")
        nc.vector.reciprocal(r2, s2)
        r2n = apool.tile([P, 1], F32, name="r2n", tag="r2n")
        nc.vector.tensor_mul(r2n, r2, self.neglam_sb[:, h : h + 1])
        p1 = apool.tile([P, S], BF16, name="p1", tag="p1", bufs=1)
        nc.vector.tensor_scalar_mul(p1, ebufs[0], r1)
        pp = apool.tile([P, S], BF16, name="pp", tag="pp", bufs=5)
        nc.vector.scalar_tensor_tensor(pp, ebufs[1], r2n, p1, op0=_mult, op1=_add)
        return pp

    def transposes(self, pp, ptbuf, qt4):
        nc = self.nc
        for g in range(2):
            pst = self.ps_tp.tile([P, 1024], BF16, name="pst", tag="tp")
            for t8 in range(8):
                kt = g * 8 + t8
                nc.tensor.transpose(
                    pst[:, t8 * P : (t8 + 1) * P],
                    pp[:, kt * P : (kt + 1) * P],
                    self.ident,
                )
            nc.vector.tensor_copy(
                ptbuf[:, g * 8 : (g + 1) * 8, qt4 * P : (qt4 + 1) * P],
                pst.rearrange("p (t q) -> p t q", t=8),
            )

    def attn_pv(self, b, h, qc, ptbuf, lo=0, width=QC):
        nc = self.nc
        if (b, h) not in self.aoT:
            self.aoT[(b, h)] = self.projpool.tile(
                [P, S], BF16, name=f"aoT{b}{h}", tag=f"aoT{b}{h}"
            )
        pso = self.ps_mm.tile([P, QC], F32, name="pso", tag="mm")
        for kt in range(ST):
            nc.tensor.matmul(
                pso[:, :width],
                lhsT=self.vbuf[b][:, kt, h * P : (h + 1) * P],
                rhs=ptbuf[:, kt, lo : lo + width],
                start=(kt == 0),
                stop=(kt == ST - 1),
            )
        nc.vector.tensor_copy(
            self.aoT[(b, h)][:, qc * QC + lo : qc * QC + lo + width],
            pso[:, :width],
        )

    # ---- o-projection piece ----
    def oproj_tt(self, out_d, b, tt):
        nc = self.nc
        ob = self.xpool.tile([P, D], F32, name="ob", tag="xc")
        for nq in range(D // 512):
            ps = self.ps_mm.tile([P, 512], F32, name="pso2", tag="mm")
            for h in range(NHL):
                nc.tensor.matmul(
                    ps,
                    lhsT=self.aoT[(b, h)][:, tt * P : (tt + 1) * P],
                    rhs=self.wo_sb[:, h, nq * 512 : (nq + 1) * 512],
                    start=(h == 0),
                    stop=(h == NHL - 1),
                )
            if nq % 2 == 0:
                nc.vector.tensor_copy(ob[:, nq * 512 : (nq + 1) * 512], ps)
            else:
                nc.scalar.copy(ob[:, nq * 512 : (nq + 1) * 512], ps)
            if nq % 2 == 1:
                nc.sync.dma_start(
                    out_d[
                        b * S + tt * P : b * S + (tt + 1) * P,
                        (nq - 1) * 512 : (nq + 1) * 512,
                    ],
                    ob[:, (nq - 1) * 512 : (nq + 1) * 512],
                )


def _proj_slot_groups(k, xt, b, h, with_v, skip_first_xc=False,
                      skip_first_wq1=False):
    """Yield emission closures for one head-slot's projections (chunks x
    4 weights, plus optionally the v projection groups)."""
    for ci in range(S // CHUNK):
        if not (skip_first_xc and ci == 0):
            yield lambda ci=ci: k.load_xc(xt, b, ci)
        for n in QKN:
            if skip_first_wq1 and ci == 0 and n == "wq1":
                continue
            yield lambda n=n, ci=ci: k.proj_qk_group(b, n, h, ci)
        if with_v:
            for s4 in range(CHUNK // P):
                yield lambda ci=ci, s4=s4: k.proj_v_group(b, ci, s4)


def _attn_head(k, b, h, bg_iter, post_pv=None):
    """Emit one head's attention, interleaving background closures (projection
    groups of the next head-slot / o-projection token tiles) between q-tiles.
    post_pv(qc) yields closures that depend on this head's PV output for chunk
    qc (the final o-projection); they are paced one per subsequent q-tile so
    their copies don't burst into a single q-tile period."""
    n_qtiles = (S // QC) * (QC // P)
    nq4 = QC // P
    last_qc = S // QC - 1
    n_bg = getattr(bg_iter, "length", 0)
    emitted = 0
    pending = []
    SKEW = 4
    prevs = []           # [(pp, ptbuf, qt4, qc)] awaiting transposes
    ptbufs = {}

    def emit_pv(qc):
        ptb = ptbufs.pop(qc)
        if post_pv is not None and qc == last_qc:
            half = QC // 2
            fns = post_pv(qc)
            k.attn_pv(b, h, qc, ptb, lo=0, width=half)
            for fn in fns[: len(fns) // 2]:
                fn()
            k.attn_pv(b, h, qc, ptb, lo=half, width=half)
            for fn in fns[len(fns) // 2 :]:
                fn()
        else:
            k.attn_pv(b, h, qc, ptb)
            if post_pv is not None:
                pending.extend(post_pv(qc))

    for i in range(n_qtiles):
        qc, qt4 = divmod(i, nq4)
        if qc not in ptbufs:
            ptbufs[qc] = k.ptpool.tile([P, KT, QC], BF16, name="ptbuf", tag="pt")
        pp = k.attn_qtile(b, h, qc, qt4, ptbufs[qc])
        # transposes run SKEW q-tiles behind so the PE never waits on the
        # DVE combine; QK(i) issues while combine(i-SKEW..i-1) are in flight
        prevs.append((pp, ptbufs[qc], qt4, qc))
        if len(prevs) > SKEW:
            ppp, ptb, pqt4, pqc = prevs.pop(0)
            k.transposes(ppp, ptb, pqt4)
            if pqt4 == nq4 - 1:
                emit_pv(pqc)
        if pending:
            pending.pop(0)()
        want = ((i + 1) * n_bg) // n_qtiles
        while emitted < want:
            next(bg_iter.it)()
            emitted += 1
    for ppp, ptb, pqt4, pqc in prevs:
        k.transposes(ppp, ptb, pqt4)
        if pqt4 == nq4 - 1:
            emit_pv(pqc)
    for fn in pending:
        fn()
    while emitted < n_bg:
        next(bg_iter.it)()
        emitted += 1


class _BG:
    def __init__(self, gens):
        items = [g for gen in gens for g in gen]
        self.it = iter(items)
        self.length = len(items)


def build_nc():
    nc = bass.Bass("TRN2", target_bir_lowering=False, debug=False)

    xt = nc.dram_tensor("xt", [D, T], BF16, kind="ExternalInput")
    wnames = ["wq1", "wk1", "wq2", "wk2", "wv"]
    w_d = {n: nc.dram_tensor(n, [P, KT * MD], BF16, kind="ExternalInput") for n in wnames}
    wo_d = nc.dram_tensor("wo", [P, NHL * D], BF16, kind="ExternalInput")
    neglam_d = nc.dram_tensor("neglam", [P, NHL], F32, kind="ExternalInput")
    out_d = nc.dram_tensor("out", [T, D], F32, kind="ExternalOutput")

    with tile.TileContext(nc) as tc:
        with (
            tc.tile_pool(name="const", bufs=1) as cpool,
            tc.tile_pool(name="proj", bufs=1) as projpool,
            tc.tile_pool(name="xchunk", bufs=2) as xpool,
            tc.tile_pool(name="attn", bufs=2) as apool,
            tc.tile_pool(name="ptp", bufs=1) as ptpool,
            tc.tile_pool(name="ps_score", bufs=2, space="PSUM") as ps_score,
            tc.tile_pool(name="ps_mm", bufs=2, space="PSUM") as ps_mm,
            tc.tile_pool(name="ps_tp", bufs=2, space="PSUM") as ps_tp,
        ):
            k = Kern(nc, tc, (cpool, projpool, xpool, apool, ptpool,
                              ps_score, ps_mm, ps_tp))
            # DMA queues drain in emission order: put the first half-chunk of
            # x and the first-used weight at the head of the line so the PE
            # starts as soon as ~2MB have landed.
            k.load_xc(xt, 0, 0, half=0)
            k.load_w(w_d, "wq1", split=True)
            k.load_xc(xt, 0, 0, half=1)
            k.load_w(w_d, "wk1", split=True)
            for n in ["wq2", "wk2", "wv"]:
                k.load_w(w_d, n)
            k.load_consts(neglam_d)

            # prologue: batch-0 head-0 projections + v(b0); the first chunk's
            # q1 projection runs as two half-width groups
            k.proj_qk_group(0, "wq1", 0, 0, half=0)
            k.proj_qk_group(0, "wq1", 0, 0, half=1)
            for fn in _proj_slot_groups(k, xt, 0, 0, with_v=True, skip_first_xc=True,
                                        skip_first_wq1=True):
                fn()
            k.load_wo(wo_d)
            # attn(b0,h0) x proj(b0,h1)
            _attn_head(k, 0, 0, _BG([_proj_slot_groups(k, xt, 0, 1, False)]))
            # attn(b0,h1) x proj(b1,h0)+v(b1)
            _attn_head(k, 0, 1, _BG([_proj_slot_groups(k, xt, 1, 0, True)]))
            # attn(b1,h0) x proj(b1,h1) + first half of oproj(b0)
            _attn_head(k, 1, 0, _BG([
                _proj_slot_groups(k, xt, 1, 1, False),
                [(lambda tt=tt: k.oproj_tt(out_d, 0, tt)) for tt in range(ST // 2)],
            ]))
            # attn(b1,h1) x oproj(b0); oproj(b1,tt) drains right after the PV
            # that completes its aoT columns, leaving almost no tail.
            def _drain_oproj_b1(qc):
                return [
                    (lambda tt=tt: k.oproj_tt(out_d, 1, tt))
                    for tt in range(4 * qc, 4 * qc + 4)
                ]

            _attn_head(
                k, 1, 1,
                _BG([[(lambda tt=tt: k.oproj_tt(out_d, 0, tt)) for tt in range(ST // 2, ST)]]),
                post_pv=_drain_oproj_b1,
            )

    _split_multi_waits(nc)
    return nc


_NC_CACHE = None


def _get_nc():
    global _NC_CACHE
    if _NC_CACHE is None:
        _NC_CACHE = build_nc()
    return _NC_CACHE



def _wlay(w_shard):
    """[MD, D] weight shard -> W.T laid out as the SBUF tile [128, KT*MD]."""
    bf = ml_dtypes.bfloat16
    wt = w_shard.T                                   # [D, MD]
    return np.ascontiguousarray(
        wt.reshape(KT, P, MD).transpose(1, 0, 2).reshape(P, KT * MD)
    ).astype(bf)


def _wolay(wo_shard):
    """[D, MD] o_w columns -> O.T laid out as the SBUF tile [128, NHL*D]."""
    bf = ml_dtypes.bfloat16
    wt = wo_shard.T                                  # [MD, D]
    return np.ascontiguousarray(
        wt.reshape(NHL, P, D).transpose(1, 0, 2).reshape(P, NHL * D)
    ).astype(bf)


def make_in_maps(inputs):
    bf = ml_dtypes.bfloat16
    x = np.asarray(inputs["x"], np.float32)
    lam = np.asarray(inputs["lambda_param"], np.float32)
    xt = np.ascontiguousarray(x.reshape(T, D).T).astype(bf)

    in_maps = []
    for c in range(N_CORES):
        hs = slice(c * MD, (c + 1) * MD)
        m = {
            "xt": xt,
            "wq1": _wlay(np.asarray(inputs["q1_w"], np.float32)[hs, :]),
            "wk1": _wlay(np.asarray(inputs["k1_w"], np.float32)[hs, :]),
            "wq2": _wlay(np.asarray(inputs["q2_w"], np.float32)[hs, :]),
            "wk2": _wlay(np.asarray(inputs["k2_w"], np.float32)[hs, :]),
            "wv": _wlay(np.asarray(inputs["v_w"], np.float32)[hs, :]),
            "wo": _wolay(np.asarray(inputs["o_w"], np.float32)[:, hs]),
            "neglam": np.tile(-lam[c * NHL : (c + 1) * NHL][None, :], (P, 1)).astype(np.float32),
        }
        in_maps.append(m)
    return in_maps


def kernel(**inputs):
    in_maps = make_in_maps(inputs)
    nc = _get_nc()
    res = bass_utils.run_bass_kernel_spmd(nc, in_maps, core_ids=list(range(N_CORES)))
    acc = np.zeros((T, D), np.float64)
    for r in res.results:
        acc += np.asarray(r["out"], np.float64)
    return acc.reshape(B, S, D).astype(np.float32)


if __name__ == "__main__":
    nc = build_nc()
    print("built OK")


# revision 45
# speedup vs baseline: 1.0085x; 1.0085x over previous
"""Differential attention (DiffAttn) Trainium2 kernel, 8-core tensor-parallel.

Reference computation (per batch b, head h):
    q1,k1,q2,k2,v = x @ W*.T          (x: [B,S,D], W: [D,D], 16 heads x 128)
    a1 = softmax(q1 k1^T / sqrt(dh)); a2 = softmax(q2 k2^T / sqrt(dh))
    out = ((a1 - lam_h * a2) @ v) @ o_w.T

Sharding: tensor-parallel over heads. Core c owns heads {2c, 2c+1} (d_model
slice 256c:256c+256 of the projection outputs).  Each core computes a partial
o-projection output over its 256 input dims; the host sums the 8 partials.

Device-side layout choices:
  - x is passed pre-transposed (xt = x.T, [D, B*S]) so projections can run
    as  out.T[m, tok] = W_shard @ x.T  with the weight shard (host
    pre-transposed) as the stationary operand -> q/k tiles land in
    [head_dim(part), token(free)] layout, which feeds QK^T directly.
  - v is produced in natural [token, dim] layout (lhsT = x.T chunks) so it can
    be the stationary operand of the PV matmul.
  - probabilities are transposed [q,k]->[k,q] on the PE in 128x128 blocks
    (batched 8 per PSUM bank); softmax row-sums ride the exp activation's
    accumulator; normalization is two DVE passes (tensor_scalar + fused STT).
  - all matmul inputs are bf16 (measured |rel err| ~5e-3 end to end vs the
    fp32 reference); PSUM accumulation is fp32; softmax stats are fp32.

Engine balance: attention is elementwise-bound (ACT exp, DVE normalize), the
projections are PE-bound.  The emission order software-pipelines them:

    proj(b0,h0)+v(b0) | attn(b0,h0) x proj(b0,h1) | attn(b0,h1) x proj(b1,h0)+v(b1)
    | attn(b1,h0) x proj(b1,h1) | attn(b1,h1) x oproj(b0) | oproj(b1)

Engines consume their instruction streams in order, so interleaved emission is
what lets the PE run projection matmuls while ACT/DVE chew on attention; the
SBUF slots are shared across batches (same tags), so Tile's WAR deps give the
handoff for free.
"""

import math

import numpy as np
import ml_dtypes

import concourse.bass as bass
import concourse.mybir as mybir
import concourse.tile as tile
from concourse import bass_utils
from concourse.masks import make_identity

BF16 = mybir.dt.bfloat16
F32 = mybir.dt.float32

P = 128           # partitions / head_dim / PE tile
D = 2048          # d_model
B = 2
S = 2048          # seq len
T = B * S         # 4096 tokens
NH = 16           # total heads
NHL = 2           # heads per core
MD = NHL * P      # per-core projection dim (256)
KT = D // P       # 16 contraction tiles over d_model
ST = S // P       # 16 token tiles per batch
N_CORES = 8
CHUNK = 512       # token chunk for projection x streaming
QC = 512          # q-chunk for the PV stage
SCALE = 1.0 / math.sqrt(P)
QKN = ["wq1", "wk1", "wq2", "wk2"]

_mult = mybir.AluOpType.mult
_add = mybir.AluOpType.add


def _split_multi_waits(nc):
    """This walrus build accepts at most ONE sync-wait per instruction
    (codegen: "Too many sync wait commands").  Tile attaches one wait per
    upstream proc, so split the extras onto same-engine NOP carriers placed
    immediately before the instruction — the engine stalls on each carrier in
    turn, which is sequentially equivalent."""
    n = 0
    for bb in nc.main_func.blocks:
        out = []
        for ins in bb.instructions:
            si = getattr(ins, "sync_info", None)
            waits = list(si.on_wait) if si is not None and si.on_wait else []
            if len(waits) > 1:
                for w in waits[:-1]:
                    n += 1
                    out.append(
                        mybir.InstNoOp(
                            name=f"{ins.name}-wsplit{n}",
                            engine=ins.engine,
                            sync_info=mybir.SyncInfo(on_wait=[w], on_update=[]),
                            bass_nofuse=True,
                        )
                    )
                si.on_wait = waits[-1:]
            out.append(ins)
        bb.instructions[:] = out


class Kern:
    """Holds pools/constants; methods emit one group of instructions each.
    The driver (build) calls them in a software-pipelined order."""

    def __init__(self, nc, tc, pools):
        self.nc = nc
        self.tc = tc
        (self.cpool, self.projpool, self.xpool, self.apool, self.ptpool,
         self.ps_score, self.ps_mm, self.ps_tp) = pools
        self.qk = {}      # (b, n, h) -> tile (slots shared across b via tags)
        self.vbuf = {}    # b -> tile
        self.aoT = {}     # (b, h) -> tile
        self.xc_cur = None
        self.w_sb = {}

    def load_w(self, w_d, n, split=False):
        t = self.cpool.tile([P, KT, MD], BF16, name=f"{n}_sb")
        src_ap = w_d[n].rearrange("p (kt m) -> p kt m", m=MD)
        if split:
            self.nc.sync.dma_start(t[:, : KT // 2], src_ap[:, : KT // 2])
            self.nc.sync.dma_start(t[:, KT // 2 :], src_ap[:, KT // 2 :])
        else:
            self.nc.sync.dma_start(t, src_ap)
        self.w_sb[n] = t

    def load_consts(self, neglam_d):
        nc = self.nc
        self.neglam_sb = self.cpool.tile([P, NHL], F32, name="neglam_sb")
        nc.sync.dma_start(self.neglam_sb, neglam_d.ap())
        self.ident = self.cpool.tile([P, P], BF16, name="ident")
        make_identity(nc, self.ident)

    def load_wo(self, wo_d):
        self.wo_sb = self.cpool.tile([P, NHL, D], BF16, name="wo_sb")
        self.nc.sync.dma_start(self.wo_sb, wo_d.rearrange("p (h n) -> p h n", n=D))

    # ---- projection pieces ----
    def load_xc(self, xt, b, ci, half=None):
        tok0 = b * S + ci * CHUNK
        if half is None:
            xc = self.xpool.tile([P, KT, CHUNK], BF16, name="xc", tag="xc")
            self.nc.sync.dma_start(
                xc, xt[:, tok0 : tok0 + CHUNK].rearrange("(kt p) n -> p kt n", p=P)
            )
            self.xc_cur = xc
        else:
            # first-chunk fast path: land each half as its own DMA so the
            # first matmuls only wait on 1MB of x
            if half == 0:
                self.xc_cur = self.xpool.tile([P, KT, CHUNK], BF16, name="xc", tag="xc")
            hw = CHUNK // 2
            self.nc.sync.dma_start(
                self.xc_cur[:, :, half * hw : (half + 1) * hw],
                xt[:, tok0 + half * hw : tok0 + (half + 1) * hw].rearrange(
                    "(kt p) n -> p kt n", p=P
                ),
            )

    def proj_qk_group(self, b, n, h, ci, half=None):
        """One (weight, head) projection for one 512-token chunk: 16 matmuls
        accumulated in PSUM, then a DVE cast into the persistent qk tile."""
        nc = self.nc
        key = (b, n, h)
        if key not in self.qk:
            self.qk[key] = self.projpool.tile(
                [P, S], BF16, name=f"{n}h{h}b{b}", tag=f"{n}h{h}"
            )
        ps = self.ps_mm.tile([P, 512], F32, name="psp", tag="mm")
        xc = self.xc_cur
        lo, sz = (0, CHUNK) if half is None else (half * (CHUNK // 2), CHUNK // 2)
        for kt in range(KT):
            nc.tensor.matmul(
                ps[:, :sz],
                lhsT=self.w_sb[n][:, kt, h * P : (h + 1) * P],
                rhs=xc[:, kt, lo : lo + sz],
                start=(kt == 0),
                stop=(kt == KT - 1),
            )
        nc.vector.tensor_copy(
            self.qk[key][:, ci * CHUNK + lo : ci * CHUNK + lo + sz], ps[:, :sz]
        )

    def proj_v_group(self, b, ci, s4):
        nc = self.nc
        if b not in self.vbuf:
            self.vbuf[b] = self.projpool.tile(
                [P, ST, MD], BF16, name=f"vbuf{b}", tag=f"vbuf{b}"
            )
        ps = self.ps_mm.tile([P, 512], F32, name="psv", tag="mm")
        xc = self.xc_cur
        for kt in range(KT):
            nc.tensor.matmul(
                ps[:, :MD],
                lhsT=xc[:, kt, s4 * P : (s4 + 1) * P],
                rhs=self.w_sb["wv"][:, kt, :],
                start=(kt == 0),
                stop=(kt == KT - 1),
            )
        nc.vector.tensor_copy(
            self.vbuf[b][:, ci * (CHUNK // P) + s4, :], ps[:, :MD]
        )

    # ---- attention pieces ----
    def attn_qtile(self, b, h, qc, qt4, ptbuf):
        nc = self.nc
        apool = self.apool
        qi = qc * QC + qt4 * P
        q1h, k1h = self.qk[(b, "wq1", h)], self.qk[(b, "wk1", h)]
        q2h, k2h = self.qk[(b, "wq2", h)], self.qk[(b, "wk2", h)]
        ebufs, rstats = [], []
        for si, (qh, kh) in enumerate([(q1h, k1h), (q2h, k2h)]):
            e = apool.tile([P, S], BF16, name=f"e{si}", tag=f"e{si}")
            ssc = apool.tile([P, 2], F32, name=f"ssc{si}", tag=f"ssc{si}")
            for hf in range(2):
                ps = self.ps_score.tile([P, 1024], F32, name="pss", tag="score")
                for j in range(2):
                    nc.tensor.matmul(
                        ps[:, j * 512 : (j + 1) * 512],
                        lhsT=qh[:, qi : qi + P],
                        rhs=kh[:, hf * 1024 + j * 512 : hf * 1024 + (j + 1) * 512],
                        start=True,
                        stop=True,
                    )
                nc.scalar.activation(
                    e[:, hf * 1024 : (hf + 1) * 1024],
                    ps,
                    mybir.ActivationFunctionType.Exp,
                    scale=SCALE,
                    accum_out=ssc[:, hf : hf + 1],
                )
            ebufs.append(e)
            rstats.append(ssc)
        s1 = apool.tile([P, 1], F32, name="s1", tag="s1")
        nc.vector.tensor_add(s1, rstats[0][:, 0:1], rstats[0][:, 1:2])
        r1 = apool.tile([P, 1], F32, name="r1", tag="r1")
        nc.vector.reciprocal(r1, s1)
        s2 = apool.tile([P, 1], F32, name="s2", tag="s2")
        nc.vector.tensor_add(s2, rstats[1][:, 0:1], rstats[1][:, 1:2])
        r2 = apool.tile([P, 1], F32, name="r2", tag="r2")
        nc.vector.reciprocal(r2, s2)
        r2n = apool.tile([P, 1], F32, name="r2n", tag="r2n")
        nc.vector.tensor_mul(r2n, r2, self.neglam_sb[:, h : h + 1])
        p1 = apool.tile([P, S], BF16, name="p1", tag="p1", bufs=1)
        nc.vector.tensor_scalar_mul(p1, ebufs[0], r1)
        pp = apool.tile([P, S], BF16, name="pp", tag="pp", bufs=4)
        nc.vector.scalar_tensor_tensor(pp, ebufs[1], r2n, p1, op0=_mult, op1=_add)
        return pp

    def transposes(self, pp, ptbuf, qt4):
        nc = self.nc
        for g in range(2):
            pst = self.ps_tp.tile([P, 1024], BF16, name="pst", tag="tp")
            for t8 in range(8):
                kt = g * 8 + t8
                nc.tensor.transpose(
                    pst[:, t8 * P : (t8 + 1) * P],
                    pp[:, kt * P : (kt + 1) * P],
                    self.ident,
                )
            nc.vector.tensor_copy(
                ptbuf[:, g * 8 : (g + 1) * 8, qt4 * P : (qt4 + 1) * P],
                pst.rearrange("p (t q) -> p t q", t=8),
            )

    def attn_pv(self, b, h, qc, ptbuf, lo=0, width=QC):
        nc = self.nc
        if (b, h) not in self.aoT:
            self.aoT[(b, h)] = self.projpool.tile(
                [P, S], BF16, name=f"aoT{b}{h}", tag=f"aoT{b}{h}"
            )
        pso = self.ps_mm.tile([P, QC], F32, name="pso", tag="mm")
        for kt in range(ST):
            nc.tensor.matmul(
                pso[:, :width],
                lhsT=self.vbuf[b][:, kt, h * P : (h + 1) * P],
                rhs=ptbuf[:, kt, lo : lo + width],
                start=(kt == 0),
                stop=(kt == ST - 1),
            )
        nc.vector.tensor_copy(
            self.aoT[(b, h)][:, qc * QC + lo : qc * QC + lo + width],
            pso[:, :width],
        )

    # ---- o-projection piece ----
    def oproj_tt(self, out_d, b, tt):
        nc = self.nc
        ob = self.xpool.tile([P, D], F32, name="ob", tag="xc")
        for nq in range(D // 512):
            ps = self.ps_mm.tile([P, 512], F32, name="pso2", tag="mm")
            for h in range(NHL):
                nc.tensor.matmul(
                    ps,
                    lhsT=self.aoT[(b, h)][:, tt * P : (tt + 1) * P],
                    rhs=self.wo_sb[:, h, nq * 512 : (nq + 1) * 512],
                    start=(h == 0),
                    stop=(h == NHL - 1),
                )
            if nq % 2 == 0:
                nc.vector.tensor_copy(ob[:, nq * 512 : (nq + 1) * 512], ps)
            else:
                nc.scalar.copy(ob[:, nq * 512 : (nq + 1) * 512], ps)
            if nq % 2 == 1:
                nc.sync.dma_start(
                    out_d[
                        b * S + tt * P : b * S + (tt + 1) * P,
                        (nq - 1) * 512 : (nq + 1) * 512,
                    ],
                    ob[:, (nq - 1) * 512 : (nq + 1) * 512],
                )


def _proj_slot_groups(k, xt, b, h, with_v, skip_first_xc=False,
                      skip_first_wq1=False):
    """Yield emission closures for one head-slot's projections (chunks x
    4 weights, plus optionally the v projection groups)."""
    for ci in range(S // CHUNK):
        if not (skip_first_xc and ci == 0):
            yield lambda ci=ci: k.load_xc(xt, b, ci)
        for n in QKN:
            if skip_first_wq1 and ci == 0 and n == "wq1":
                continue
            yield lambda n=n, ci=ci: k.proj_qk_group(b, n, h, ci)
        if with_v:
            for s4 in range(CHUNK // P):
                yield lambda ci=ci, s4=s4: k.proj_v_group(b, ci, s4)


def _attn_head(k, b, h, bg_iter, post_pv=None):
    """Emit one head's attention, interleaving background closures (projection
    groups of the next head-slot / o-projection token tiles) between q-tiles.
    post_pv(qc) yields closures that depend on this head's PV output for chunk
    qc (the final o-projection); they are paced one per subsequent q-tile so
    their copies don't burst into a single q-tile period."""
    n_qtiles = (S // QC) * (QC // P)
    nq4 = QC // P
    last_qc = S // QC - 1
    n_bg = getattr(bg_iter, "length", 0)
    emitted = 0
    pending = []
    SKEW = 3
    prevs = []           # [(pp, ptbuf, qt4, qc)] awaiting transposes
    pv_queue = []
    ptbufs = {}

    def emit_pv(qc):
        ptb = ptbufs.pop(qc)
        if post_pv is not None and qc == last_qc:
            half = QC // 2
            fns = post_pv(qc)
            k.attn_pv(b, h, qc, ptb, lo=0, width=half)
            for fn in fns[: len(fns) // 2]:
                fn()
            k.attn_pv(b, h, qc, ptb, lo=half, width=half)
            for fn in fns[len(fns) // 2 :]:
                fn()
        else:
            k.attn_pv(b, h, qc, ptb)
            if post_pv is not None:
                pending.extend(post_pv(qc))

    for i in range(n_qtiles):
        qc, qt4 = divmod(i, nq4)
        if qc not in ptbufs:
            ptbufs[qc] = k.ptpool.tile([P, KT, QC], BF16, name="ptbuf", tag="pt")
        pp = k.attn_qtile(b, h, qc, qt4, ptbufs[qc])
        # PV is skewed one q-tile past its last transposes so the PE never
        # waits on the DVE ptbuf copies either
        if pv_queue:
            emit_pv(pv_queue.pop(0))
        # transposes run SKEW q-tiles behind so the PE never waits on the
        # DVE combine; QK(i) issues while combine(i-SKEW..i-1) are in flight
        prevs.append((pp, ptbufs[qc], qt4, qc))
        if len(prevs) > SKEW:
            ppp, ptb, pqt4, pqc = prevs.pop(0)
            k.transposes(ppp, ptb, pqt4)
            if pqt4 == nq4 - 1:
                pv_queue.append(pqc)
        if pending:
            pending.pop(0)()
        want = ((i + 1) * n_bg) // n_qtiles
        while emitted < want:
            next(bg_iter.it)()
            emitted += 1
    for ppp, ptb, pqt4, pqc in prevs:
        k.transposes(ppp, ptb, pqt4)
        if pqt4 == nq4 - 1:
            pv_queue.append(pqc)
    for pqc in pv_queue:
        emit_pv(pqc)
    for fn in pending:
        fn()
    while emitted < n_bg:
        next(bg_iter.it)()
        emitted += 1


class _BG:
    def __init__(self, gens):
        items = [g for gen in gens for g in gen]
        self.it = iter(items)
        self.length = len(items)


def build_nc():
    nc = bass.Bass("TRN2", target_bir_lowering=False, debug=False)

    xt = nc.dram_tensor("xt", [D, T], BF16, kind="ExternalInput")
    wnames = ["wq1", "wk1", "wq2", "wk2", "wv"]
    w_d = {n: nc.dram_tensor(n, [P, KT * MD], BF16, kind="ExternalInput") for n in wnames}
    wo_d = nc.dram_tensor("wo", [P, NHL * D], BF16, kind="ExternalInput")
    neglam_d = nc.dram_tensor("neglam", [P, NHL], F32, kind="ExternalInput")
    out_d = nc.dram_tensor("out", [T, D], F32, kind="ExternalOutput")

    with tile.TileContext(nc) as tc:
        with (
            tc.tile_pool(name="const", bufs=1) as cpool,
            tc.tile_pool(name="proj", bufs=1) as projpool,
            tc.tile_pool(name="xchunk", bufs=2) as xpool,
            tc.tile_pool(name="attn", bufs=2) as apool,
            tc.tile_pool(name="ptp", bufs=1) as ptpool,
            tc.tile_pool(name="ps_score", bufs=2, space="PSUM") as ps_score,
            tc.tile_pool(name="ps_mm", bufs=2, space="PSUM") as ps_mm,
            tc.tile_pool(name="ps_tp", bufs=2, space="PSUM") as ps_tp,
        ):
            k = Kern(nc, tc, (cpool, projpool, xpool, apool, ptpool,
                              ps_score, ps_mm, ps_tp))
            # DMA queues drain in emission order: put the first half-chunk of
            # x and the first-used weight at the head of the line so the PE
            # starts as soon as ~2MB have landed.
            k.load_xc(xt, 0, 0, half=0)
            k.load_w(w_d, "wq1", split=True)
            k.load_xc(xt, 0, 0, half=1)
            for n in ["wk1", "wq2", "wk2", "wv"]:
                k.load_w(w_d, n)
            k.load_consts(neglam_d)

            # prologue: batch-0 head-0 projections + v(b0); the first chunk's
            # q1 projection runs as two half-width groups
            k.proj_qk_group(0, "wq1", 0, 0, half=0)
            k.proj_qk_group(0, "wq1", 0, 0, half=1)
            for fn in _proj_slot_groups(k, xt, 0, 0, with_v=True, skip_first_xc=True,
                                        skip_first_wq1=True):
                fn()
            k.load_wo(wo_d)
            # attn(b0,h0) x proj(b0,h1)
            _attn_head(k, 0, 0, _BG([_proj_slot_groups(k, xt, 0, 1, False)]))
            # attn(b0,h1) x proj(b1,h0)+v(b1)
            _attn_head(k, 0, 1, _BG([_proj_slot_groups(k, xt, 1, 0, True)]))
            # attn(b1,h0) x proj(b1,h1) + first half of oproj(b0)
            _attn_head(k, 1, 0, _BG([
                _proj_slot_groups(k, xt, 1, 1, False),
                [(lambda tt=tt: k.oproj_tt(out_d, 0, tt)) for tt in range(ST // 2)],
            ]))
            # attn(b1,h1) x oproj(b0); oproj(b1,tt) drains right after the PV
            # that completes its aoT columns, leaving almost no tail.
            def _drain_oproj_b1(qc):
                return [
                    (lambda tt=tt: k.oproj_tt(out_d, 1, tt))
                    for tt in range(4 * qc, 4 * qc + 4)
                ]

            _attn_head(
                k, 1, 1,
                _BG([[(lambda tt=tt: k.oproj_tt(out_d, 0, tt)) for tt in range(ST // 2, ST)]]),
                post_pv=_drain_oproj_b1,
            )

    _split_multi_waits(nc)
    return nc


_NC_CACHE = None


def _get_nc():
    global _NC_CACHE
    if _NC_CACHE is None:
        _NC_CACHE = build_nc()
    return _NC_CACHE



def _wlay(w_shard):
    """[MD, D] weight shard -> W.T laid out as the SBUF tile [128, KT*MD]."""
    bf = ml_dtypes.bfloat16
    wt = w_shard.T                                   # [D, MD]
    return np.ascontiguousarray(
        wt.reshape(KT, P, MD).transpose(1, 0, 2).reshape(P, KT * MD)
    ).astype(bf)


def _wolay(wo_shard):
    """[D, MD] o_w columns -> O.T laid out as the SBUF tile [128, NHL*D]."""
    bf = ml_dtypes.bfloat16
    wt = wo_shard.T                                  # [MD, D]
    return np.ascontiguousarray(
        wt.reshape(NHL, P, D).transpose(1, 0, 2).reshape(P, NHL * D)
    ).astype(bf)


def make_in_maps(inputs):
    bf = ml_dtypes.bfloat16
    x = np.asarray(inputs["x"], np.float32)
    lam = np.asarray(inputs["lambda_param"], np.float32)
    xt = np.ascontiguousarray(x.reshape(T, D).T).astype(bf)

    in_maps = []
    for c in range(N_CORES):
        hs = slice(c * MD, (c + 1) * MD)
        m = {
            "xt": xt,
            "wq1": _wlay(np.asarray(inputs["q1_w"], np.float32)[hs, :]),
            "wk1": _wlay(np.asarray(inputs["k1_w"], np.float32)[hs, :]),
            "wq2": _wlay(np.asarray(inputs["q2_w"], np.float32)[hs, :]),
            "wk2": _wlay(np.asarray(inputs["k2_w"], np.float32)[hs, :]),
            "wv": _wlay(np.asarray(inputs["v_w"], np.float32)[hs, :]),
            "wo": _wolay(np.asarray(inputs["o_w"], np.float32)[:, hs]),
            "neglam": np.tile(-lam[c * NHL : (c + 1) * NHL][None, :], (P, 1)).astype(np.float32),
        }
        in_maps.append(m)
    return in_maps


def kernel(**inputs):
    in_maps = make_in_maps(inputs)
    nc = _get_nc()
    res = bass_utils.run_bass_kernel_spmd(nc, in_maps, core_ids=list(range(N_CORES)))
    acc = np.zeros((T, D), np.float64)
    for r in res.results:
        acc += np.asarray(r["out"], np.float64)
    return acc.reshape(B, S, D).astype(np.float32)


if __name__ == "__main__":
    nc = build_nc()
    print("built OK")


# revision 49
# speedup vs baseline: 1.0176x; 1.0090x over previous
"""Differential attention (DiffAttn) Trainium2 kernel, 8-core tensor-parallel.

Reference computation (per batch b, head h):
    q1,k1,q2,k2,v = x @ W*.T          (x: [B,S,D], W: [D,D], 16 heads x 128)
    a1 = softmax(q1 k1^T / sqrt(dh)); a2 = softmax(q2 k2^T / sqrt(dh))
    out = ((a1 - lam_h * a2) @ v) @ o_w.T

Sharding: tensor-parallel over heads. Core c owns heads {2c, 2c+1} (d_model
slice 256c:256c+256 of the projection outputs).  Each core computes a partial
o-projection output over its 256 input dims; the host sums the 8 partials.

Device-side layout choices:
  - x is passed pre-transposed (xt = x.T, [D, B*S]) so projections can run
    as  out.T[m, tok] = W_shard @ x.T  with the weight shard (host
    pre-transposed) as the stationary operand -> q/k tiles land in
    [head_dim(part), token(free)] layout, which feeds QK^T directly.
  - v is produced in natural [token, dim] layout (lhsT = x.T chunks) so it can
    be the stationary operand of the PV matmul.
  - probabilities are transposed [q,k]->[k,q] on the PE in 128x128 blocks
    (batched 8 per PSUM bank); softmax row-sums ride the exp activation's
    accumulator; normalization is two DVE passes (tensor_scalar + fused STT).
  - all matmul inputs are bf16 (measured |rel err| ~5e-3 end to end vs the
    fp32 reference); PSUM accumulation is fp32; softmax stats are fp32.

Engine balance: attention is elementwise-bound (ACT exp, DVE normalize), the
projections are PE-bound.  The emission order software-pipelines them:

    proj(b0,h0)+v(b0) | attn(b0,h0) x proj(b0,h1) | attn(b0,h1) x proj(b1,h0)+v(b1)
    | attn(b1,h0) x proj(b1,h1) | attn(b1,h1) x oproj(b0) | oproj(b1)

Engines consume their instruction streams in order, so interleaved emission is
what lets the PE run projection matmuls while ACT/DVE chew on attention; the
SBUF slots are shared across batches (same tags), so Tile's WAR deps give the
handoff for free.
"""

import math

import numpy as np
import ml_dtypes

import concourse.bass as bass
import concourse.mybir as mybir
import concourse.tile as tile
from concourse import bass_utils
from concourse.masks import make_identity

BF16 = mybir.dt.bfloat16
F32 = mybir.dt.float32

P = 128           # partitions / head_dim / PE tile
D = 2048          # d_model
B = 2
S = 2048          # seq len
T = B * S         # 4096 tokens
NH = 16           # total heads
NHL = 2           # heads per core
MD = NHL * P      # per-core projection dim (256)
KT = D // P       # 16 contraction tiles over d_model
ST = S // P       # 16 token tiles per batch
N_CORES = 8
CHUNK = 512       # token chunk for projection x streaming
QC = 512          # q-chunk for the PV stage
SCALE = 1.0 / math.sqrt(P)
QKN = ["wq1", "wk1", "wq2", "wk2"]

_mult = mybir.AluOpType.mult
_add = mybir.AluOpType.add


def _split_multi_waits(nc):
    """This walrus build accepts at most ONE sync-wait per instruction
    (codegen: "Too many sync wait commands").  Tile attaches one wait per
    upstream proc, so split the extras onto same-engine NOP carriers placed
    immediately before the instruction — the engine stalls on each carrier in
    turn, which is sequentially equivalent."""
    n = 0
    for bb in nc.main_func.blocks:
        out = []
        for ins in bb.instructions:
            si = getattr(ins, "sync_info", None)
            waits = list(si.on_wait) if si is not None and si.on_wait else []
            if len(waits) > 1:
                for w in waits[:-1]:
                    n += 1
                    out.append(
                        mybir.InstNoOp(
                            name=f"{ins.name}-wsplit{n}",
                            engine=ins.engine,
                            sync_info=mybir.SyncInfo(on_wait=[w], on_update=[]),
                            bass_nofuse=True,
                        )
                    )
                si.on_wait = waits[-1:]
            out.append(ins)
        bb.instructions[:] = out


class Kern:
    """Holds pools/constants; methods emit one group of instructions each.
    The driver (build) calls them in a software-pipelined order."""

    def __init__(self, nc, tc, pools):
        self.nc = nc
        self.tc = tc
        (self.cpool, self.projpool, self.xpool, self.apool, self.ptpool,
         self.ps_score, self.ps_mm, self.ps_tp) = pools
        self.qk = {}      # (b, n, h) -> tile (slots shared across b via tags)
        self.vbuf = {}    # b -> tile
        self.aoT = {}     # (b, h) -> tile
        self.xc_cur = None
        self.w_sb = {}

    def load_w(self, w_d, n, split=False):
        t = self.cpool.tile([P, KT, MD], BF16, name=f"{n}_sb")
        src_ap = w_d[n].rearrange("p (kt m) -> p kt m", m=MD)
        if split:
            self.nc.sync.dma_start(t[:, : KT // 2], src_ap[:, : KT // 2])
            self.nc.sync.dma_start(t[:, KT // 2 :], src_ap[:, KT // 2 :])
        else:
            self.nc.sync.dma_start(t, src_ap)
        self.w_sb[n] = t

    def load_consts(self, neglam_d):
        nc = self.nc
        self.neglam_sb = self.cpool.tile([P, NHL], F32, name="neglam_sb")
        nc.sync.dma_start(self.neglam_sb, neglam_d.ap())
        self.ident = self.cpool.tile([P, P], BF16, name="ident")
        make_identity(nc, self.ident)

    def load_wo(self, wo_d):
        self.wo_sb = self.cpool.tile([P, NHL, D], BF16, name="wo_sb")
        self.nc.sync.dma_start(self.wo_sb, wo_d.rearrange("p (h n) -> p h n", n=D))

    # ---- projection pieces ----
    def load_xc(self, xt, b, ci, half=None):
        tok0 = b * S + ci * CHUNK
        if half is None:
            xc = self.xpool.tile([P, KT, CHUNK], BF16, name="xc", tag="xc")
            self.nc.sync.dma_start(
                xc, xt[:, tok0 : tok0 + CHUNK].rearrange("(kt p) n -> p kt n", p=P)
            )
            self.xc_cur = xc
        else:
            # first-chunk fast path: land each half as its own DMA so the
            # first matmuls only wait on 1MB of x
            if half == 0:
                self.xc_cur = self.xpool.tile([P, KT, CHUNK], BF16, name="xc", tag="xc")
            hw = CHUNK // 2
            self.nc.sync.dma_start(
                self.xc_cur[:, :, half * hw : (half + 1) * hw],
                xt[:, tok0 + half * hw : tok0 + (half + 1) * hw].rearrange(
                    "(kt p) n -> p kt n", p=P
                ),
            )

    def proj_qk_group(self, b, n, h, ci, half=None):
        """One (weight, head) projection for one 512-token chunk: 16 matmuls
        accumulated in PSUM, then a DVE cast into the persistent qk tile."""
        nc = self.nc
        key = (b, n, h)
        if key not in self.qk:
            self.qk[key] = self.projpool.tile(
                [P, S], BF16, name=f"{n}h{h}b{b}", tag=f"{n}h{h}"
            )
        ps = self.ps_mm.tile([P, 512], F32, name="psp", tag="mm")
        xc = self.xc_cur
        lo, sz = (0, CHUNK) if half is None else (half * (CHUNK // 2), CHUNK // 2)
        for kt in range(KT):
            nc.tensor.matmul(
                ps[:, :sz],
                lhsT=self.w_sb[n][:, kt, h * P : (h + 1) * P],
                rhs=xc[:, kt, lo : lo + sz],
                start=(kt == 0),
                stop=(kt == KT - 1),
            )
        nc.vector.tensor_copy(
            self.qk[key][:, ci * CHUNK + lo : ci * CHUNK + lo + sz], ps[:, :sz]
        )

    def proj_v_group(self, b, ci, s4):
        nc = self.nc
        if b not in self.vbuf:
            self.vbuf[b] = self.projpool.tile(
                [P, ST, MD], BF16, name=f"vbuf{b}", tag=f"vbuf{b}"
            )
        ps = self.ps_mm.tile([P, 512], F32, name="psv", tag="mm")
        xc = self.xc_cur
        for kt in range(KT):
            nc.tensor.matmul(
                ps[:, :MD],
                lhsT=xc[:, kt, s4 * P : (s4 + 1) * P],
                rhs=self.w_sb["wv"][:, kt, :],
                start=(kt == 0),
                stop=(kt == KT - 1),
            )
        nc.vector.tensor_copy(
            self.vbuf[b][:, ci * (CHUNK // P) + s4, :], ps[:, :MD]
        )

    # ---- attention pieces ----
    def attn_qtile(self, b, h, qc, qt4, ptbuf, mid=None):
        nc = self.nc
        apool = self.apool
        qi = qc * QC + qt4 * P
        q1h, k1h = self.qk[(b, "wq1", h)], self.qk[(b, "wk1", h)]
        q2h, k2h = self.qk[(b, "wq2", h)], self.qk[(b, "wk2", h)]
        ebufs, rstats = [], []
        for si, (qh, kh) in enumerate([(q1h, k1h), (q2h, k2h)]):
            if si == 1 and mid is not None:
                # softmax-2's first matmul reuses softmax-1's PSUM slot and
                # must wait for exp1 (measured ~2us); slot background work
                # here so the PE has something to chew on meanwhile
                mid()
            e = apool.tile([P, S], BF16, name=f"e{si}", tag=f"e{si}")
            ssc = apool.tile([P, 2], F32, name=f"ssc{si}", tag=f"ssc{si}")
            for hf in range(2):
                ps = self.ps_score.tile([P, 1024], F32, name="pss", tag="score")
                for j in range(2):
                    nc.tensor.matmul(
                        ps[:, j * 512 : (j + 1) * 512],
                        lhsT=qh[:, qi : qi + P],
                        rhs=kh[:, hf * 1024 + j * 512 : hf * 1024 + (j + 1) * 512],
                        start=True,
                        stop=True,
                    )
                nc.scalar.activation(
                    e[:, hf * 1024 : (hf + 1) * 1024],
                    ps,
                    mybir.ActivationFunctionType.Exp,
                    scale=SCALE,
                    accum_out=ssc[:, hf : hf + 1],
                )
            ebufs.append(e)
            rstats.append(ssc)
        s1 = apool.tile([P, 1], F32, name="s1", tag="s1")
        nc.vector.tensor_add(s1, rstats[0][:, 0:1], rstats[0][:, 1:2])
        r1 = apool.tile([P, 1], F32, name="r1", tag="r1")
        nc.vector.reciprocal(r1, s1)
        s2 = apool.tile([P, 1], F32, name="s2", tag="s2")
        nc.vector.tensor_add(s2, rstats[1][:, 0:1], rstats[1][:, 1:2])
        r2 = apool.tile([P, 1], F32, name="r2", tag="r2")
        nc.vector.reciprocal(r2, s2)
        r2n = apool.tile([P, 1], F32, name="r2n", tag="r2n")
        nc.vector.tensor_mul(r2n, r2, self.neglam_sb[:, h : h + 1])
        p1 = apool.tile([P, S], BF16, name="p1", tag="p1", bufs=1)
        nc.vector.tensor_scalar_mul(p1, ebufs[0], r1)
        pp = apool.tile([P, S], BF16, name="pp", tag="pp", bufs=4)
        nc.vector.scalar_tensor_tensor(pp, ebufs[1], r2n, p1, op0=_mult, op1=_add)
        return pp

    def transposes(self, pp, ptbuf, qt4):
        nc = self.nc
        for g in range(2):
            pst = self.ps_tp.tile([P, 1024], BF16, name="pst", tag="tp")
            for t8 in range(8):
                kt = g * 8 + t8
                nc.tensor.transpose(
                    pst[:, t8 * P : (t8 + 1) * P],
                    pp[:, kt * P : (kt + 1) * P],
                    self.ident,
                )
            nc.vector.tensor_copy(
                ptbuf[:, g * 8 : (g + 1) * 8, qt4 * P : (qt4 + 1) * P],
                pst.rearrange("p (t q) -> p t q", t=8),
            )

    def attn_pv(self, b, h, qc, ptbuf, lo=0, width=QC):
        nc = self.nc
        if (b, h) not in self.aoT:
            self.aoT[(b, h)] = self.projpool.tile(
                [P, S], BF16, name=f"aoT{b}{h}", tag=f"aoT{b}{h}"
            )
        pso = self.ps_mm.tile([P, QC], F32, name="pso", tag="mm")
        for kt in range(ST):
            nc.tensor.matmul(
                pso[:, :width],
                lhsT=self.vbuf[b][:, kt, h * P : (h + 1) * P],
                rhs=ptbuf[:, kt, lo : lo + width],
                start=(kt == 0),
                stop=(kt == ST - 1),
            )
        nc.vector.tensor_copy(
            self.aoT[(b, h)][:, qc * QC + lo : qc * QC + lo + width],
            pso[:, :width],
        )

    # ---- o-projection piece ----
    def oproj_tt(self, out_d, b, tt):
        nc = self.nc
        ob = self.xpool.tile([P, D], F32, name="ob", tag="xc")
        for nq in range(D // 512):
            ps = self.ps_mm.tile([P, 512], F32, name="pso2", tag="mm")
            for h in range(NHL):
                nc.tensor.matmul(
                    ps,
                    lhsT=self.aoT[(b, h)][:, tt * P : (tt + 1) * P],
                    rhs=self.wo_sb[:, h, nq * 512 : (nq + 1) * 512],
                    start=(h == 0),
                    stop=(h == NHL - 1),
                )
            if nq % 2 == 0:
                nc.vector.tensor_copy(ob[:, nq * 512 : (nq + 1) * 512], ps)
            else:
                nc.scalar.copy(ob[:, nq * 512 : (nq + 1) * 512], ps)
            if nq % 2 == 1:
                nc.sync.dma_start(
                    out_d[
                        b * S + tt * P : b * S + (tt + 1) * P,
                        (nq - 1) * 512 : (nq + 1) * 512,
                    ],
                    ob[:, (nq - 1) * 512 : (nq + 1) * 512],
                )


def _proj_slot_groups(k, xt, b, h, with_v, skip_first_xc=False,
                      skip_first_wq1=False):
    """Yield emission closures for one head-slot's projections (chunks x
    4 weights, plus optionally the v projection groups)."""
    for ci in range(S // CHUNK):
        if not (skip_first_xc and ci == 0):
            yield lambda ci=ci: k.load_xc(xt, b, ci)
        for n in QKN:
            if skip_first_wq1 and ci == 0 and n == "wq1":
                continue
            yield lambda n=n, ci=ci: k.proj_qk_group(b, n, h, ci)
        if with_v:
            for s4 in range(CHUNK // P):
                yield lambda ci=ci, s4=s4: k.proj_v_group(b, ci, s4)


def _attn_head(k, b, h, bg_iter, post_pv=None):
    """Emit one head's attention, interleaving background closures (projection
    groups of the next head-slot / o-projection token tiles) between q-tiles.
    post_pv(qc) yields closures that depend on this head's PV output for chunk
    qc (the final o-projection); they are paced one per subsequent q-tile so
    their copies don't burst into a single q-tile period."""
    n_qtiles = (S // QC) * (QC // P)
    nq4 = QC // P
    last_qc = S // QC - 1
    n_bg = getattr(bg_iter, "length", 0)
    emitted = 0
    pending = []
    SKEW = 3
    prevs = []           # [(pp, ptbuf, qt4, qc)] awaiting transposes
    ptbufs = {}

    def emit_pv(qc):
        ptb = ptbufs.pop(qc)
        if post_pv is not None and qc == last_qc:
            half = QC // 2
            fns = post_pv(qc)
            k.attn_pv(b, h, qc, ptb, lo=0, width=half)
            for fn in fns[: len(fns) // 2]:
                fn()
            k.attn_pv(b, h, qc, ptb, lo=half, width=half)
            for fn in fns[len(fns) // 2 :]:
                fn()
        else:
            k.attn_pv(b, h, qc, ptb)
            if post_pv is not None:
                pending.extend(post_pv(qc))

    for i in range(n_qtiles):
        qc, qt4 = divmod(i, nq4)
        if qc not in ptbufs:
            ptbufs[qc] = k.ptpool.tile([P, KT, QC], BF16, name="ptbuf", tag="pt")
        def mid():
            nonlocal emitted
            if pending:
                pending.pop(0)()
            elif emitted < n_bg:
                next(bg_iter.it)()
                emitted += 1

        pp = k.attn_qtile(b, h, qc, qt4, ptbufs[qc], mid=mid)
        # transposes run SKEW q-tiles behind so the PE never waits on the
        # DVE combine; QK(i) issues while combine(i-SKEW..i-1) are in flight
        prevs.append((pp, ptbufs[qc], qt4, qc))
        if len(prevs) > SKEW:
            ppp, ptb, pqt4, pqc = prevs.pop(0)
            k.transposes(ppp, ptb, pqt4)
            if pqt4 == nq4 - 1:
                emit_pv(pqc)
        if pending:
            pending.pop(0)()
        want = ((i + 1) * n_bg) // n_qtiles
        while emitted < want:
            next(bg_iter.it)()
            emitted += 1
    for ppp, ptb, pqt4, pqc in prevs:
        k.transposes(ppp, ptb, pqt4)
        if pqt4 == nq4 - 1:
            emit_pv(pqc)
    for fn in pending:
        fn()
    while emitted < n_bg:
        next(bg_iter.it)()
        emitted += 1


class _BG:
    def __init__(self, gens):
        items = [g for gen in gens for g in gen]
        self.it = iter(items)
        self.length = len(items)


def build_nc():
    nc = bass.Bass("TRN2", target_bir_lowering=False, debug=False)

    xt = nc.dram_tensor("xt", [D, T], BF16, kind="ExternalInput")
    wnames = ["wq1", "wk1", "wq2", "wk2", "wv"]
    w_d = {n: nc.dram_tensor(n, [P, KT * MD], BF16, kind="ExternalInput") for n in wnames}
    wo_d = nc.dram_tensor("wo", [P, NHL * D], BF16, kind="ExternalInput")
    neglam_d = nc.dram_tensor("neglam", [P, NHL], F32, kind="ExternalInput")
    out_d = nc.dram_tensor("out", [T, D], F32, kind="ExternalOutput")

    with tile.TileContext(nc) as tc:
        with (
            tc.tile_pool(name="const", bufs=1) as cpool,
            tc.tile_pool(name="proj", bufs=1) as projpool,
            tc.tile_pool(name="xchunk", bufs=2) as xpool,
            tc.tile_pool(name="attn", bufs=2) as apool,
            tc.tile_pool(name="ptp", bufs=1) as ptpool,
            tc.tile_pool(name="ps_score", bufs=2, space="PSUM") as ps_score,
            tc.tile_pool(name="ps_mm", bufs=2, space="PSUM") as ps_mm,
            tc.tile_pool(name="ps_tp", bufs=2, space="PSUM") as ps_tp,
        ):
            k = Kern(nc, tc, (cpool, projpool, xpool, apool, ptpool,
                              ps_score, ps_mm, ps_tp))
            # DMA queues drain in emission order: put the first half-chunk of
            # x and the first-used weight at the head of the line so the PE
            # starts as soon as ~2MB have landed.
            k.load_xc(xt, 0, 0, half=0)
            k.load_w(w_d, "wq1", split=True)
            k.load_xc(xt, 0, 0, half=1)
            for n in ["wk1", "wq2", "wk2", "wv"]:
                k.load_w(w_d, n)
            k.load_consts(neglam_d)

            # prologue: batch-0 head-0 projections + v(b0); the first chunk's
            # q1 projection runs as two half-width groups
            k.proj_qk_group(0, "wq1", 0, 0, half=0)
            k.proj_qk_group(0, "wq1", 0, 0, half=1)
            for fn in _proj_slot_groups(k, xt, 0, 0, with_v=True, skip_first_xc=True,
                                        skip_first_wq1=True):
                fn()
            k.load_wo(wo_d)
            # attn(b0,h0) x proj(b0,h1)
            _attn_head(k, 0, 0, _BG([_proj_slot_groups(k, xt, 0, 1, False)]))
            # attn(b0,h1) x proj(b1,h0)+v(b1)
            _attn_head(k, 0, 1, _BG([_proj_slot_groups(k, xt, 1, 0, True)]))
            # attn(b1,h0) x proj(b1,h1) + first half of oproj(b0)
            _attn_head(k, 1, 0, _BG([
                _proj_slot_groups(k, xt, 1, 1, False),
                [(lambda tt=tt: k.oproj_tt(out_d, 0, tt)) for tt in range(ST // 2)],
            ]))
            # attn(b1,h1) x oproj(b0); oproj(b1,tt) drains right after the PV
            # that completes its aoT columns, leaving almost no tail.
            def _drain_oproj_b1(qc):
                return [
                    (lambda tt=tt: k.oproj_tt(out_d, 1, tt))
                    for tt in range(4 * qc, 4 * qc + 4)
                ]

            _attn_head(
                k, 1, 1,
                _BG([[(lambda tt=tt: k.oproj_tt(out_d, 0, tt)) for tt in range(ST // 2, ST)]]),
                post_pv=_drain_oproj_b1,
            )

    _split_multi_waits(nc)
    return nc


_NC_CACHE = None


def _get_nc():
    global _NC_CACHE
    if _NC_CACHE is None:
        _NC_CACHE = build_nc()
    return _NC_CACHE



def _wlay(w_shard):
    """[MD, D] weight shard -> W.T laid out as the SBUF tile [128, KT*MD]."""
    bf = ml_dtypes.bfloat16
    wt = w_shard.T                                   # [D, MD]
    return np.ascontiguousarray(
        wt.reshape(KT, P, MD).transpose(1, 0, 2).reshape(P, KT * MD)
    ).astype(bf)


def _wolay(wo_shard):
    """[D, MD] o_w columns -> O.T laid out as the SBUF tile [128, NHL*D]."""
    bf = ml_dtypes.bfloat16
    wt = wo_shard.T                                  # [MD, D]
    return np.ascontiguousarray(
        wt.reshape(NHL, P, D).transpose(1, 0, 2).reshape(P, NHL * D)
    ).astype(bf)


def make_in_maps(inputs):
    bf = ml_dtypes.bfloat16
    x = np.asarray(inputs["x"], np.float32)
    lam = np.asarray(inputs["lambda_param"], np.float32)
    xt = np.ascontiguousarray(x.reshape(T, D).T).astype(bf)

    in_maps = []
    for c in range(N_CORES):
        hs = slice(c * MD, (c + 1) * MD)
        m = {
            "xt": xt,
            "wq1": _wlay(np.asarray(inputs["q1_w"], np.float32)[hs, :]),
            "wk1": _wlay(np.asarray(inputs["k1_w"], np.float32)[hs, :]),
            "wq2": _wlay(np.asarray(inputs["q2_w"], np.float32)[hs, :]),
            "wk2": _wlay(np.asarray(inputs["k2_w"], np.float32)[hs, :]),
            "wv": _wlay(np.asarray(inputs["v_w"], np.float32)[hs, :]),
            "wo": _wolay(np.asarray(inputs["o_w"], np.float32)[:, hs]),
            "neglam": np.tile(-lam[c * NHL : (c + 1) * NHL][None, :], (P, 1)).astype(np.float32),
        }
        in_maps.append(m)
    return in_maps


def kernel(**inputs):
    in_maps = make_in_maps(inputs)
    nc = _get_nc()
    res = bass_utils.run_bass_kernel_spmd(nc, in_maps, core_ids=list(range(N_CORES)))
    acc = np.zeros((T, D), np.float64)
    for r in res.results:
        acc += np.asarray(r["out"], np.float64)
    return acc.reshape(B, S, D).astype(np.float32)


if __name__ == "__main__":
    nc = build_nc()
    print("built OK")
